# revision 28
# baseline (speedup 1.0000x reference)
"""Trainium2 Bass kernel for AbsoluteSinusoidal2DPE logits.

Math (flattened, N = H*W = 1024, D = 512):
    logits[b] = q[b] @ e^T + e @ (k[b] + e)^T          # [N, N] per batch

Key structure: the embed table is separable, e[(i,j), :] = eh[i, :] + ew[j, :].
With the rank-64 basis E2 = [eh; ew] ([64, D]) and the 0/1 selection matrix
sel[m, (a,b)] = [m == a] + [m == 32 + b] ([64, N]), the logits factor exactly:

    ABt = E2 @ q[b]^T                      # [64, N]
    CD  = E2 @ (k[b] + e)^T                # [64, N]  (e folded into k on host)
    logits[b] = sel^T @ CD + ABt^T @ sel   # [N, N]

The expansion is emitted as ONE K=128 matmul per 512-col output half:
stacking LT = [sel; ABt] ([128, N]) and RT = [CD; sel] ([128, N]) gives
    out_tile = LT[:, rows]^T @ RT[:, cols] = sel^T CD + ABt^T sel
in a single pass — half the PE cycles of the two-matmul (K=64) form.

Dtypes: q and (k+e) ship as float8_e3m4 (the 2e-2 rel-err gate is an absolute
budget of ~6.5 given |logits| in [325, 1115]; fp8 inputs measure ~8e-3).
Output ships as bfloat16 (adds <= 2^-9 pointwise rel err) and is upcast to
f32 on host: per-batch DMA drops to ~3.1 MB (1 MB in + 2 MB out) from the
5.2 MB of the f32-out variant, and HBM wire time is the floor.

Scheduling notes:
  - LT/RT are double-buffered per batch (sel halves pre-filled once, the
    Pool engine fills the second set) so batch b+1's projections never
    serialize against batch b's expansion reads.
  - PSUM: 4 single-bank front tiles (pk0/pk1/pa0/pa1) + 4 single-bank
    expansion tiles (po0..po3); half-width copies keep the
    matmul->copy->reuse round-trip short.
  - Batch b+1's load issues before batch b's stores in ring order; the
    first batch's kt half rides first (behind the tiny e2t) so the pk
    matmuls start at quarter-transfer.
  - The PE p-state resets on idle, so the warm loop is sized to bridge
    the gap from program start to the first data-dependent matmul.

Sharding: batch dim (16) data-parallel over 8 cores, 2 batches/core.
"""

import numpy as np

B, H, W, D = 16, 32, 32, 512
N = H * W            # 1024
NCORES = 8
BPC = B // NCORES    # batches per core
P = 128              # partitions
KO = D // P          # 4 contraction chunks
NT = N // P          # 8 output row tiles
R = 64               # separable basis rank (32 rows + 32 cols)
NPAIR = NT // 2      # stores go out in row-tile pairs

_PROG = None  # cached bass program, reused across kernel() calls
# "f8e3": e3m4 inputs (rel err ~9e-3 unfolded / ~1.4e-2 folded)
# "f8e4": e4m3 inputs + DoubleRow front matmuls at 0.5 cyc/row
#         (rel err ~1.7e-2, requires UNFOLD)
# "f16":  fp16 inputs (~6e-4), 2B/elem
INPUT_DT = "f8e4"
# UNFOLD: ship cde = E2 @ e^T separately (f32) instead of folding e into k
# on host; costs one extra const DMA, halves the input-quantization error
UNFOLD = True


def _input_np_dt():
    import ml_dtypes
    if INPUT_DT == "f8e3":
        return ml_dtypes.float8_e3m4
    if INPUT_DT == "f8e4":
        return ml_dtypes.float8_e4m3
    return np.float16


def _build_program(n_batches: int = BPC, loop_reps: int = 0,
                   prewarm: bool = True,
                   copy_eng: str = "vavavavavavavava",
                   sync_stores: int = 4,
                   n_po: int = 4, single_stores_last: bool = False,
                   warm_n: int = 16, warm_w: int = 64):
    """n_batches > BPC repeats the batch loop (cycling the same DRAM data);
    loop_reps > 0 wraps the whole body in a For_i hardware loop (timing
    instrument; prewarm is skipped there). The real kernel uses defaults.

    copy_eng: per-half-tile engine for the 16 PSUM->SBUF output copies of a
    batch: 'v' = DVE, 'a' = ACT ('g' = Pool is NOT legal here: GPSIMD
    cannot access PSUM, the BIR verifier rejects it).
    sync_stores: how many of each batch's 4 pair-stores issue on the SP
    (sync) ring; the rest use the ACT ring.
    """
    import contextlib
    import concourse.mybir as mybir
    import concourse.tile as tile
    from concourse import bacc

    F32 = mybir.dt.float32
    F32R = mybir.dt.float32r
    BF16 = mybir.dt.bfloat16
    FIN = {"f8e3": mybir.dt.float8e3, "f8e4": mybir.dt.float8e4,
           "f16": mybir.dt.float16}[INPUT_DT]
    dbl_row = INPUT_DT == "f8e4"

    nc = bacc.Bacc()
    # inputs are host-swizzled to partition-major [P, 2, KO, N]: slot 0 is
    # k^T (or (k+e)^T when folded), slot 1 is q^T, so each partition's whole
    # per-batch load is one contiguous 8KB DRAM block -> optimal DMA lines
    qk_d = nc.dram_tensor("qk", [BPC, P, 2, KO, N], FIN, kind="ExternalInput")
    e2t_d = nc.dram_tensor("e2t", [P, KO, R], FIN, kind="ExternalInput")
    sel_d = nc.dram_tensor("sel", [R, N], FIN, kind="ExternalInput")
    cde_d = (nc.dram_tensor("cde", [R, N], F32, kind="ExternalInput")
             if UNFOLD else None)
    out_d = nc.dram_tensor("out", [BPC, NPAIR, P, 2, N], BF16,
                           kind="ExternalOutput")

    with tile.TileContext(nc) as tc:
        with (
            tc.tile_pool(name="cst", bufs=1) as cst,
            tc.tile_pool(name="inp", bufs=2) as inp,
            tc.tile_pool(name="outp", bufs=4) as outp,
            tc.tile_pool(name="ps", bufs=1, space="PSUM") as psp,
        ):
          loop_cm = tc.For_i(0, loop_reps, 1) if loop_reps else contextlib.nullcontext()
          with loop_cm:
            e2t = cst.tile([P, KO, R], FIN, name="e2t")
            selb = cst.tile([R, N], FIN, name="selb")
            cde = cst.tile([R, N], F32, name="cde") if UNFOLD else None
            # LT = [sel; abt], RT = [cd; sel], double-buffered per batch;
            # the sel halves are filled once, abt/cd rewritten per batch
            LTs = [cst.tile([P, N], F32R, name=f"LT{i}") for i in range(2)]
            RTs = [cst.tile([P, N], F32R, name=f"RT{i}") for i in range(2)]

            qk_tiles = {}

            def emit_load(b):
                qk_tiles[b] = inp.tile([P, 2, KO, N], FIN, tag="qk", name="qk")
                nc.sync.dma_start(qk_tiles[b][:], qk_d[b % BPC])

            # first batch: e2t (tiny) first, then the kt half in two
            # quarter-loads so the pk matmuls start at quarter-transfer
            qk0 = qk_tiles[0] = inp.tile([P, 2, KO, N], FIN, tag="qk", name="qk")
            # e2t + sel ride the gpsimd SWDGE ring: the HWDGE generator is a
            # single shared unit (~625ns per descriptor set), so keeping the
            # startup-critical qk quarters alone on it starts compute sooner
            nc.gpsimd.dma_start(e2t[:], e2t_d[:, :, :])
            nc.sync.dma_start(qk0[:, 0, :, 0:512], qk_d[0, :, 0, :, 0:512])
            nc.sync.dma_start(qk0[:, 1, :, 0:512], qk_d[0, :, 1, :, 0:512])
            nc.sync.dma_start(qk0[:, 0, :, 512:N], qk_d[0, :, 0, :, 512:N])
            nc.sync.dma_start(qk0[:, 1, :, 512:N], qk_d[0, :, 1, :, 512:N])
            nc.gpsimd.dma_start(selb[:], sel_d[:, :])
            if UNFOLD:
                nc.gpsimd.dma_start(cde[:], cde_d[:, :])
            # sel halves: batch-0 set on DVE/ACT, batch-1 set on Pool
            nc.vector.tensor_copy(LTs[0][0:R, :], selb[:])
            nc.scalar.copy(RTs[0][R:P, :], selb[:])
            nc.gpsimd.tensor_copy(LTs[1][0:R, :], selb[:])
            nc.gpsimd.tensor_copy(RTs[1][R:P, :], selb[:])

            if prewarm and not loop_reps:
                # PE pre-warm: dummy matmuls while the first input DMAs are
                # in flight. The p-state clock resets on idle, so the warm
                # loop is sized to run until the first real matmul is ready.
                warm = cst.tile([P, 128], F32R, name="warm")
                nc.vector.memset(warm[:].bitcast(F32), 0.0)
                warm_ps = psp.tile([P, 512], F32, tag="po0", name="warm_ps")
                for _ in range(warm_n):
                    nc.tensor.matmul(warm_ps[0:warm_w, 0:warm_w],
                                     warm[:, 0:warm_w], warm[:, 0:warm_w],
                                     start=True, stop=True)

            def emit_front(b):
                """In-projections for batch b: fills RT[0:64] (cd) and
                LT[64:128] (abt). Column-half groups run back to back with
                each half's copy emitted as soon as its group closes, so the
                copies overlap the remaining groups' matmuls."""
                qk = qk_tiles[b]
                LT, RT = LTs[b % 2], RTs[b % 2]
                # h0 groups (k then q) first: the expansion's h0 matmuls of
                # row tiles 0-3 depend only on cd-h0 + abt-h0, so they can
                # interleave with the h1 groups still running on PE
                def front_mms(ps, kq, hs):
                    """Projection matmuls into psum rows 0:64. e4m3 inputs
                    use DoubleRow perf mode: two 128-row contraction chunks
                    per instruction at 0.5 cycles/row."""
                    if dbl_row:
                        for g in range(2):
                            ks = slice(2 * g, 2 * g + 2)
                            nc.tensor.matmul(
                                ps[0:R, :], e2t[:, ks, :], qk[:, kq, ks, hs],
                                start=(g == 0), stop=(g == 1),
                                perf_mode=mybir.MatmulPerfMode.DoubleRow)
                    else:
                        for ko in range(KO):
                            nc.tensor.matmul(ps[0:R, :], e2t[:, ko],
                                             qk[:, kq, ko, hs],
                                             start=(ko == 0),
                                             stop=(ko == KO - 1))

                # front PSUM tiles live in banks 4-7 ("ps4".."ps7"); when
                # n_po == 8 the expansion's row tiles 4-7 reuse those banks
                # after the front copies drain
                nfb = 1 if n_po == 6 else 2
                for h in range(2):
                    hs = slice(h * 512, (h + 1) * 512)
                    t = f"ps{4 + (h % nfb)}"
                    pk = psp.tile([P, 512], F32, tag=t, name=t)
                    front_mms(pk, 0, hs)
                    if UNFOLD:
                        nc.vector.tensor_add(RT[0:R, hs], pk[0:R, :],
                                             cde[:, hs])
                    else:
                        nc.vector.tensor_copy(RT[0:R, hs], pk[0:R, :])
                    t = f"ps{6 + (h % nfb)}"
                    pa = psp.tile([P, 512], F32, tag=t, name=t)
                    front_mms(pa, 1, hs)
                    nc.scalar.copy(LT[R:P, hs], pa[0:R, :])

            def emit_exp(b, last_batch):
                """Expansion of batch b: one K=128 matmul per 512-col half
                into a single-bank PSUM tile, half-width copy to bf16,
                paired stores."""
                LT, RT = LTs[b % 2], RTs[b % 2]
                for nt in range(NT):
                    if nt % 2 == 0:
                        ob = outp.tile([P, 2, N], BF16, tag="ob", name="ob")
                    lhs = LT[:, nt * P:(nt + 1) * P]
                    for h in range(2):
                        hs = slice(h * 512, (h + 1) * 512)
                        pid = (2 * nt + h) % n_po
                        t = f"po{pid}" if pid < 4 else f"ps{pid}"
                        po = psp.tile([P, 512], F32, tag=t, name=t)
                        nc.tensor.matmul(po[:], lhs, RT[:, hs],
                                         start=True, stop=True)
                        eng = {"v": nc.vector.tensor_copy,
                               "a": nc.scalar.copy,
                               "g": nc.gpsimd.tensor_copy}[copy_eng[2 * nt + h]]
                        eng(ob[:, nt % 2, hs], po[:])
                    dst = out_d[b % BPC, nt // 2]
                    if single_stores_last == 2 or (last_batch and single_stores_last):
                        # finer stores on the last batch: each row tile ships
                        # as soon as its copies land, shortening the drain
                        ring = (nc.sync, nc.scalar)[nt % 2]
                        ring.dma_start(dst[:, nt % 2], ob[:, nt % 2])
                    elif nt % 2 == 1:
                        np_ = nt // 2
                        if last_batch and np_ == NPAIR - 1:
                            # split the final store across both HWDGE rings
                            nc.scalar.dma_start(dst[:, 0], ob[:, 0])
                            nc.sync.dma_start(dst[:, 1], ob[:, 1])
                        elif np_ < sync_stores:
                            nc.sync.dma_start(dst, ob[:])
                        else:
                            nc.scalar.dma_start(dst, ob[:])

            for b in range(n_batches):
                emit_front(b)
                if b + 1 < n_batches:
                    # prefetch next batch's input now so its DMA precedes
                    # this batch's stores in ring order
                    emit_load(b + 1)
                emit_exp(b, b == n_batches - 1)

    nc.compile()
    return nc


def _make_consts(embed: np.ndarray):
    """Host-side prep of the tiny batch-independent operands."""
    ef = embed.reshape(N, D).astype(np.float32)
    eh = ef[0:N:W]                      # embed[:, 0, :]   [32, D]
    ew = ef[0:W] - ef[0]                # embed[0, :, :] - embed[0, 0, :]
    e2 = np.concatenate([eh, ew], axis=0)            # [64, D]
    e2t = np.ascontiguousarray(e2.T).astype(_input_np_dt())  # [D, 64]
    e2t = np.ascontiguousarray(
        e2t.reshape(KO, P, R).transpose(1, 0, 2))  # [P, KO, 64]
    sel = np.zeros((R, N), np.float32)
    idx = np.arange(N)
    sel[idx // W, idx] = 1.0
    sel[W + idx % W, idx] = 1.0
    sel = sel.astype(_input_np_dt())   # 0/1: exact in fp8
    # cde = E2 @ e^T in host f32 (the device adds it to the k-projection)
    cde = np.ascontiguousarray(e2 @ ef.T) if UNFOLD else None
    return e2t, sel, cde


def kernel(q: np.ndarray, k: np.ndarray, embed: np.ndarray) -> np.ndarray:
    global _PROG
    from concourse import bass_utils

    q = np.asarray(q)
    k = np.asarray(k)
    embed = np.asarray(embed)
    assert q.shape == (B, H, W, D) and k.shape == (B, H, W, D)
    assert embed.shape == (H, W, D)

    qf = q.reshape(B, N, D).astype(np.float32, copy=False)
    kf = k.reshape(B, N, D).astype(np.float32)
    if not UNFOLD:
        # fold the position table into k on host: CD = E2 @ (k + e)^T
        kf = kf + embed.reshape(1, N, D)

    # [B, D, N] low-precision transposes (RNE cast, matches device
    # numerics), then swizzled partition-major to [B, P, 2, KO, N]
    dt = _input_np_dt()
    qt = np.ascontiguousarray(qf.transpose(0, 2, 1)).astype(dt)
    kt = np.ascontiguousarray(kf.transpose(0, 2, 1)).astype(dt)
    qt = np.ascontiguousarray(
        qt.reshape(B, KO, P, N).transpose(0, 2, 1, 3))
    kt = np.ascontiguousarray(
        kt.reshape(B, KO, P, N).transpose(0, 2, 1, 3))
    qk = np.stack([kt, qt], axis=2)    # [B, P, 2, KO, N]
    e2t, sel, cde = _make_consts(embed)

    if _PROG is None:
        _PROG = _build_program()
    nc = _PROG

    in_maps = []
    for c in range(NCORES):
        sl = slice(c * BPC, (c + 1) * BPC)
        m = {"qk": qk[sl], "e2t": e2t, "sel": sel}
        if UNFOLD:
            m["cde"] = cde
        in_maps.append(m)

    res = bass_utils.run_bass_kernel_spmd(nc, in_maps, core_ids=list(range(NCORES)))
    outs = [r["out"] for r in res.results]   # each [BPC, NPAIR, P, 2, N] bf16
    full = np.concatenate(outs, axis=0).astype(np.float32)
    # [B, NPAIR, P, 2, N] -> rows r = pair*256 + j*128 + p
    full = full.transpose(0, 1, 3, 2, 4).reshape(B, N, N)
    return np.ascontiguousarray(full.reshape(B, H, W, H, W))


# revision 41
# speedup vs baseline: 1.0439x; 1.0439x over previous
"""Trainium2 Bass kernel for AbsoluteSinusoidal2DPE logits.

Math (flattened, N = H*W = 1024, D = 512):
    logits[b] = q[b] @ e^T + e @ (k[b] + e)^T          # [N, N] per batch

Key structure: the embed table is separable, e[(i,j), :] = eh[i, :] + ew[j, :].
With the rank-64 basis E2 = [eh; ew] ([64, D]) and the 0/1 selection matrix
sel[m, (a,b)] = [m == a] + [m == 32 + b] ([64, N]), the logits factor exactly:

    ABt = E2 @ q[b]^T                      # [64, N]
    CD  = E2 @ (k[b] + e)^T                # [64, N]  (e folded into k on host)
    logits[b] = sel^T @ CD + ABt^T @ sel   # [N, N]

The expansion is emitted as ONE K=128 matmul per 512-col output half:
stacking LT = [sel; ABt] ([128, N]) and RT = [CD; sel] ([128, N]) gives
    out_tile = LT[:, rows]^T @ RT[:, cols] = sel^T CD + ABt^T sel
in a single pass — half the PE cycles of the two-matmul (K=64) form.

Dtypes: q and (k+e) ship as float8_e3m4 (the 2e-2 rel-err gate is an absolute
budget of ~6.5 given |logits| in [325, 1115]; fp8 inputs measure ~8e-3).
Output ships as bfloat16 (adds <= 2^-9 pointwise rel err) and is upcast to
f32 on host: per-batch DMA drops to ~3.1 MB (1 MB in + 2 MB out) from the
5.2 MB of the f32-out variant, and HBM wire time is the floor.

Scheduling notes:
  - LT/RT are double-buffered per batch (sel halves pre-filled once, the
    Pool engine fills the second set) so batch b+1's projections never
    serialize against batch b's expansion reads.
  - PSUM: 4 single-bank front tiles (pk0/pk1/pa0/pa1) + 4 single-bank
    expansion tiles (po0..po3); half-width copies keep the
    matmul->copy->reuse round-trip short.
  - Batch b+1's load issues before batch b's stores in ring order; the
    first batch's kt half rides first (behind the tiny e2t) so the pk
    matmuls start at quarter-transfer.
  - The PE p-state resets on idle, so the warm loop is sized to bridge
    the gap from program start to the first data-dependent matmul.

Sharding: batch dim (16) data-parallel over 8 cores, 2 batches/core.
"""

import numpy as np

B, H, W, D = 16, 32, 32, 512
N = H * W            # 1024
NCORES = 8
BPC = B // NCORES    # batches per core
P = 128              # partitions
KO = D // P          # 4 contraction chunks
NT = N // P          # 8 output row tiles
R = 64               # separable basis rank (32 rows + 32 cols)
NPAIR = NT // 2      # stores go out in row-tile pairs

_PROG = None  # cached bass program, reused across kernel() calls
# "f8e3": e3m4 inputs (rel err ~9e-3 unfolded / ~1.4e-2 folded)
# "f8e4": e4m3 inputs + DoubleRow front matmuls at 0.5 cyc/row
#         (rel err ~1.7e-2, requires UNFOLD)
# "f16":  fp16 inputs (~6e-4), 2B/elem
INPUT_DT = "f8e3"
# UNFOLD: ship cde = E2 @ e^T separately (f32) instead of folding e into k
# on host; costs one extra const DMA, halves the input-quantization error
UNFOLD = True


def _input_np_dt():
    import ml_dtypes
    if INPUT_DT == "f8e3":
        return ml_dtypes.float8_e3m4
    if INPUT_DT == "f8e4":
        return ml_dtypes.float8_e4m3
    return np.float16


def _build_program(n_batches: int = BPC, loop_reps: int = 0,
                   prewarm: bool = True,
                   copy_eng: str = "vavavavavavavava",
                   sync_stores: int = 4,
                   n_po: int = 4, single_stores_last: bool = True,
                   warm_n: int = 16, warm_w: int = 64):
    """n_batches > BPC repeats the batch loop (cycling the same DRAM data);
    loop_reps > 0 wraps the whole body in a For_i hardware loop (timing
    instrument; prewarm is skipped there). The real kernel uses defaults.

    copy_eng: per-half-tile engine for the 16 PSUM->SBUF output copies of a
    batch: 'v' = DVE, 'a' = ACT ('g' = Pool is NOT legal here: GPSIMD
    cannot access PSUM, the BIR verifier rejects it).
    sync_stores: how many of each batch's 4 pair-stores issue on the SP
    (sync) ring; the rest use the ACT ring.
    """
    import contextlib
    import concourse.mybir as mybir
    import concourse.tile as tile
    from concourse import bacc

    F32 = mybir.dt.float32
    F32R = mybir.dt.float32r
    BF16 = mybir.dt.bfloat16
    FIN = {"f8e3": mybir.dt.float8e3, "f8e4": mybir.dt.float8e4,
           "f16": mybir.dt.float16}[INPUT_DT]
    dbl_row = INPUT_DT == "f8e4"

    nc = bacc.Bacc()
    # inputs are host-swizzled to partition-major [P, 2, KO, N]: slot 0 is
    # k^T (or (k+e)^T when folded), slot 1 is q^T, so each partition's whole
    # per-batch load is one contiguous 8KB DRAM block -> optimal DMA lines
    qk_d = nc.dram_tensor("qk", [BPC, P, 2, KO, N], FIN, kind="ExternalInput")
    e2t_d = nc.dram_tensor("e2t", [P, KO, R], FIN, kind="ExternalInput")
    sel_d = nc.dram_tensor("sel", [R, N], FIN, kind="ExternalInput")
    cde_d = (nc.dram_tensor("cde", [R, N], F32, kind="ExternalInput")
             if UNFOLD else None)
    out_d = nc.dram_tensor("out", [BPC, NPAIR, P, 2, N], BF16,
                           kind="ExternalOutput")

    with tile.TileContext(nc) as tc:
        with (
            tc.tile_pool(name="cst", bufs=1) as cst,
            tc.tile_pool(name="inp", bufs=2) as inp,
            tc.tile_pool(name="outp", bufs=4) as outp,
            tc.tile_pool(name="ps", bufs=1, space="PSUM") as psp,
        ):
          loop_cm = tc.For_i(0, loop_reps, 1) if loop_reps else contextlib.nullcontext()
          with loop_cm:
            e2t = cst.tile([P, KO, R], FIN, name="e2t")
            selb = cst.tile([R, N], FIN, name="selb")
            cde = cst.tile([R, N], F32, name="cde") if UNFOLD else None
            # LT = [sel; abt], RT = [cd; sel], double-buffered per batch;
            # the sel halves are filled once, abt/cd rewritten per batch
            LTs = [cst.tile([P, N], F32R, name=f"LT{i}") for i in range(2)]
            RTs = [cst.tile([P, N], F32R, name=f"RT{i}") for i in range(2)]

            qk_tiles = {}

            def emit_load(b):
                qk_tiles[b] = inp.tile([P, 2, KO, N], FIN, tag="qk", name="qk")
                nc.sync.dma_start(qk_tiles[b][:], qk_d[b % BPC])

            # first batch: e2t (tiny) first, then the kt half in two
            # quarter-loads so the pk matmuls start at quarter-transfer
            qk0 = qk_tiles[0] = inp.tile([P, 2, KO, N], FIN, tag="qk", name="qk")
            # e2t + sel ride the gpsimd SWDGE ring: the HWDGE generator is a
            # single shared unit (~625ns per descriptor set), so keeping the
            # startup-critical qk quarters alone on it starts compute sooner
            nc.gpsimd.dma_start(e2t[:], e2t_d[:, :, :])
            nc.sync.dma_start(qk0[:, 0, :, 0:512], qk_d[0, :, 0, :, 0:512])
            nc.sync.dma_start(qk0[:, 1, :, 0:512], qk_d[0, :, 1, :, 0:512])
            nc.sync.dma_start(qk0[:, 0, :, 512:N], qk_d[0, :, 0, :, 512:N])
            nc.sync.dma_start(qk0[:, 1, :, 512:N], qk_d[0, :, 1, :, 512:N])
            nc.gpsimd.dma_start(selb[:], sel_d[:, :])
            if UNFOLD:
                nc.gpsimd.dma_start(cde[:], cde_d[:, :])
            # sel halves: batch-0 set on DVE/ACT, batch-1 set on Pool
            nc.vector.tensor_copy(LTs[0][0:R, :], selb[:])
            nc.scalar.copy(RTs[0][R:P, :], selb[:])
            nc.gpsimd.tensor_copy(LTs[1][0:R, :], selb[:])
            nc.gpsimd.tensor_copy(RTs[1][R:P, :], selb[:])

            if prewarm and not loop_reps:
                # PE pre-warm: dummy matmuls while the first input DMAs are
                # in flight. The p-state clock resets on idle, so the warm
                # loop is sized to run until the first real matmul is ready.
                warm = cst.tile([P, 128], F32R, name="warm")
                nc.vector.memset(warm[:].bitcast(F32), 0.0)
                warm_ps = psp.tile([P, 512], F32, tag="po0", name="warm_ps")
                for _ in range(warm_n):
                    nc.tensor.matmul(warm_ps[0:warm_w, 0:warm_w],
                                     warm[:, 0:warm_w], warm[:, 0:warm_w],
                                     start=True, stop=True)

            def emit_front(b):
                """In-projections for batch b: fills RT[0:64] (cd) and
                LT[64:128] (abt). Column-half groups run back to back with
                each half's copy emitted as soon as its group closes, so the
                copies overlap the remaining groups' matmuls."""
                qk = qk_tiles[b]
                LT, RT = LTs[b % 2], RTs[b % 2]
                # h0 groups (k then q) first: the expansion's h0 matmuls of
                # row tiles 0-3 depend only on cd-h0 + abt-h0, so they can
                # interleave with the h1 groups still running on PE
                def front_mms(ps, kq, hs):
                    """Projection matmuls into psum rows 0:64. e4m3 inputs
                    use DoubleRow perf mode: two 128-row contraction chunks
                    per instruction at 0.5 cycles/row."""
                    if dbl_row:
                        for g in range(2):
                            ks = slice(2 * g, 2 * g + 2)
                            nc.tensor.matmul(
                                ps[0:R, :], e2t[:, ks, :], qk[:, kq, ks, hs],
                                start=(g == 0), stop=(g == 1),
                                perf_mode=mybir.MatmulPerfMode.DoubleRow)
                    else:
                        for ko in range(KO):
                            nc.tensor.matmul(ps[0:R, :], e2t[:, ko],
                                             qk[:, kq, ko, hs],
                                             start=(ko == 0),
                                             stop=(ko == KO - 1))

                # front PSUM tiles live in banks 4-7 ("ps4".."ps7"); when
                # n_po == 8 the expansion's row tiles 4-7 reuse those banks
                # after the front copies drain
                nfb = 1 if n_po == 6 else 2
                for h in range(2):
                    hs = slice(h * 512, (h + 1) * 512)
                    t = f"ps{4 + (h % nfb)}"
                    pk = psp.tile([P, 512], F32, tag=t, name=t)
                    front_mms(pk, 0, hs)
                    if UNFOLD:
                        nc.vector.tensor_add(RT[0:R, hs], pk[0:R, :],
                                             cde[:, hs])
                    else:
                        nc.vector.tensor_copy(RT[0:R, hs], pk[0:R, :])
                    t = f"ps{6 + (h % nfb)}"
                    pa = psp.tile([P, 512], F32, tag=t, name=t)
                    front_mms(pa, 1, hs)
                    nc.scalar.copy(LT[R:P, hs], pa[0:R, :])

            def emit_exp(b, last_batch):
                """Expansion of batch b: one K=128 matmul per 512-col half
                into a single-bank PSUM tile, half-width copy to bf16,
                paired stores."""
                LT, RT = LTs[b % 2], RTs[b % 2]
                for nt in range(NT):
                    if nt % 2 == 0:
                        ob = outp.tile([P, 2, N], BF16, tag="ob", name="ob")
                    lhs = LT[:, nt * P:(nt + 1) * P]
                    for h in range(2):
                        hs = slice(h * 512, (h + 1) * 512)
                        pid = (2 * nt + h) % n_po
                        t = f"po{pid}" if pid < 4 else f"ps{pid}"
                        po = psp.tile([P, 512], F32, tag=t, name=t)
                        nc.tensor.matmul(po[:], lhs, RT[:, hs],
                                         start=True, stop=True)
                        eng = {"v": nc.vector.tensor_copy,
                               "a": nc.scalar.copy,
                               "g": nc.gpsimd.tensor_copy}[copy_eng[2 * nt + h]]
                        eng(ob[:, nt % 2, hs], po[:])
                    dst = out_d[b % BPC, nt // 2]
                    if single_stores_last == 2 or (last_batch and single_stores_last):
                        # finer stores on the last batch: each row tile ships
                        # as soon as its copies land, shortening the drain
                        ring = (nc.sync, nc.scalar)[nt % 2]
                        ring.dma_start(dst[:, nt % 2], ob[:, nt % 2])
                    elif nt % 2 == 1:
                        np_ = nt // 2
                        if last_batch and np_ == NPAIR - 1:
                            # split the final store across both HWDGE rings
                            nc.scalar.dma_start(dst[:, 0], ob[:, 0])
                            nc.sync.dma_start(dst[:, 1], ob[:, 1])
                        elif np_ < sync_stores:
                            nc.sync.dma_start(dst, ob[:])
                        else:
                            nc.scalar.dma_start(dst, ob[:])

            for b in range(n_batches):
                emit_front(b)
                if b + 1 < n_batches:
                    # prefetch next batch's input now so its DMA precedes
                    # this batch's stores in ring order
                    emit_load(b + 1)
                emit_exp(b, b == n_batches - 1)

    nc.compile()
    return nc


def _make_consts(embed: np.ndarray):
    """Host-side prep of the tiny batch-independent operands."""
    ef = embed.reshape(N, D).astype(np.float32)
    eh = ef[0:N:W]                      # embed[:, 0, :]   [32, D]
    ew = ef[0:W] - ef[0]                # embed[0, :, :] - embed[0, 0, :]
    e2 = np.concatenate([eh, ew], axis=0)            # [64, D]
    e2t = np.ascontiguousarray(e2.T).astype(_input_np_dt())  # [D, 64]
    e2t = np.ascontiguousarray(
        e2t.reshape(KO, P, R).transpose(1, 0, 2))  # [P, KO, 64]
    sel = np.zeros((R, N), np.float32)
    idx = np.arange(N)
    sel[idx // W, idx] = 1.0
    sel[W + idx % W, idx] = 1.0
    sel = sel.astype(_input_np_dt())   # 0/1: exact in fp8
    # cde = E2 @ e^T in host f32 (the device adds it to the k-projection)
    cde = np.ascontiguousarray(e2 @ ef.T) if UNFOLD else None
    return e2t, sel, cde


def kernel(q: np.ndarray, k: np.ndarray, embed: np.ndarray) -> np.ndarray:
    global _PROG
    from concourse import bass_utils

    q = np.asarray(q)
    k = np.asarray(k)
    embed = np.asarray(embed)
    assert q.shape == (B, H, W, D) and k.shape == (B, H, W, D)
    assert embed.shape == (H, W, D)

    qf = q.reshape(B, N, D).astype(np.float32, copy=False)
    kf = k.reshape(B, N, D).astype(np.float32)
    if not UNFOLD:
        # fold the position table into k on host: CD = E2 @ (k + e)^T
        kf = kf + embed.reshape(1, N, D)

    # [B, D, N] low-precision transposes (RNE cast, matches device
    # numerics), then swizzled partition-major to [B, P, 2, KO, N]
    dt = _input_np_dt()
    qt = np.ascontiguousarray(qf.transpose(0, 2, 1)).astype(dt)
    kt = np.ascontiguousarray(kf.transpose(0, 2, 1)).astype(dt)
    qt = np.ascontiguousarray(
        qt.reshape(B, KO, P, N).transpose(0, 2, 1, 3))
    kt = np.ascontiguousarray(
        kt.reshape(B, KO, P, N).transpose(0, 2, 1, 3))
    qk = np.stack([kt, qt], axis=2)    # [B, P, 2, KO, N]
    e2t, sel, cde = _make_consts(embed)

    if _PROG is None:
        _PROG = _build_program()
    nc = _PROG

    in_maps = []
    for c in range(NCORES):
        sl = slice(c * BPC, (c + 1) * BPC)
        m = {"qk": qk[sl], "e2t": e2t, "sel": sel}
        if UNFOLD:
            m["cde"] = cde
        in_maps.append(m)

    res = bass_utils.run_bass_kernel_spmd(nc, in_maps, core_ids=list(range(NCORES)))
    outs = [r["out"] for r in res.results]   # each [BPC, NPAIR, P, 2, N] bf16
    full = np.concatenate(outs, axis=0).astype(np.float32)
    # [B, NPAIR, P, 2, N] -> rows r = pair*256 + j*128 + p
    full = full.transpose(0, 1, 3, 2, 4).reshape(B, N, N)
    return np.ascontiguousarray(full.reshape(B, H, W, H, W))
